# revision 10
# baseline (speedup 1.0000x reference)
"""DKVMN Trainium2 kernel — 8-core data-parallel over batch.

Per core (B_local = 2 sequences):
  * code2vec path-triple embeddings gathered via dma_gather TRANSPOSE mode
    from bf16 tables -> fullT[k, (l,p)] tiles feeding the PE directly as the
    stationary operand of z = full @ W_pt.
  * cw = tanh(z) @ W_att as a fused DVE scalar_tensor_tensor (accum_out),
    producing cw[p, l] columns; softmax over L is then a free-dim softmax.
  * code_vecT = sum_p attn * fullT on DVE: attn is DMA-flattened then
    DMA-replicated across partitions (broadcast src AP), mul + segmented
    reduce over the resident gather tiles.
  * Time scan in [d, (b,m)] layout: e/a arrive as per-partition scalars from
    TRANSPOSED e/a matmuls; w arrives as a DMA-replicated texture; the read
    is recovered from the update's accum (read = accum(w*e*Mv)/e, with e
    clamped away from 0).
All biases in this model instance are zero (checked at build time; nonzero
biases fold into ACT bias columns / K=1 rank-1 matmuls when present).
"""

import os
import numpy as np
import ml_dtypes

import concourse.bass as bass
import concourse.bacc as bacc
import concourse.mybir as mybir
import concourse.tile as tile
from concourse.bass_utils import run_bass_kernel_spmd
from concourse.library_config import mlp as mlp_lib

BF16 = ml_dtypes.bfloat16

# Problem shapes (hardcoded per harness contract).
B, L = 16, 128
NUM_C = 100
D = 128          # DIM_S
M = 64           # SIZE_M
E = 128          # EMB
P = 128          # MAX_CODE_LEN (paths per position)
NODE_ROWS = 10002
PATH_ROWS = 10002
CORES = 8
BPC = B // CORES  # batches per core = 2

F32 = mybir.dt.float32
BF = mybir.dt.bfloat16
I16 = mybir.dt.int16
AX = mybir.AxisListType
OP = mybir.AluOpType
AF = mybir.ActivationFunctionType

_CACHE = {}


def _wrap_idx(flat):
    """dma_gather index layout: wrapped[p, s] = flat[s*16+p], tiled to 128 partitions."""
    flat = np.ascontiguousarray(flat.astype(np.int16))
    n = flat.shape[0]
    assert n % 16 == 0
    w = flat.reshape(n // 16, 16).T  # [16, n/16]
    return np.tile(w, (8, 1))  # [128, n/16]


def _bcast_rows(ap, nparts):
    """Partition-broadcast src AP for DMA replication (step-0 partition dim)."""
    return bass.AP(tensor=ap.tensor, offset=ap.offset,
                   ap=[[0, nparts]] + list(ap.ap[1:]))


def _emit(nc, t, has_bias):
    tc = t["tc"]

    const = tc.alloc_tile_pool(name="const", bufs=1)
    gpool = tc.alloc_tile_pool(name="gpool", bufs=7)
    tanhp = tc.alloc_tile_pool(name="tanhp", bufs=3)
    scrp = tc.alloc_tile_pool(name="scrp", bufs=2)
    perb = tc.alloc_tile_pool(name="perb", bufs=2)
    prodp = tc.alloc_tile_pool(name="prodp", bufs=2)
    zps = tc.alloc_tile_pool(name="zps", bufs=2, space="PSUM")
    sps = tc.alloc_tile_pool(name="sps", bufs=4, space="PSUM")

    nc.gpsimd.load_library(mlp_lib)

    # ---- constants to SBUF ----
    wpt_sb = const.tile([128, 3, 384], BF)     # W_pt rows chunk c -> [:, c, :]
    nc.sync.dma_start(wpt_sb[:], t["wpt"].ap().rearrange("(c p) j -> p c j", p=128))
    wea_sb = const.tile([128, 4, 256], BF)
    nc.sync.dma_start(wea_sb[:], t["wea"].ap().rearrange("(c p) j -> p c j", p=128))
    wf_sb = const.tile([128, 2, 128], BF)
    nc.sync.dma_start(wf_sb[:], t["wf"].ap().rearrange("(c p) j -> p c j", p=128))
    wp_sb = const.tile([128, 10], BF)
    nc.sync.dma_start(wp_sb[:], t["wp"].ap())
    mkT_sb = const.tile([128, 64], BF)
    nc.sync.dma_start(mkT_sb[:], t["mkT"].ap())
    watt_rep = const.tile([128, 384], BF)      # W_att row, host-replicated
    nc.sync.dma_start(watt_rep[:], t["watt_rep"].ap())
    idg_sb = const.tile([128, 3 * 2048], I16)
    nc.sync.dma_start(idg_sb[:], t["idx3"].ap())
    idqr_sb = const.tile([128, 32], I16)
    nc.sync.dma_start(idqr_sb[:], t["idxqr"].ap())
    ident_bf = const.tile([128, 128], BF)
    nc.sync.dma_start(ident_bf[:], t["ident_bf"].ap())
    mv = const.tile([128, 128], F32)           # [d, (b,m)] running memory state
    nc.sync.dma_start(mv[:], t["mv0T2"].ap())
    if has_bias["any"]:
        ones_bf = const.tile([1, 128], BF)
        nc.sync.dma_start(ones_bf[:], t["ones_row"].ap())
        brow_sb = const.tile([1, 384], BF)      # b_pt row
        nc.sync.dma_start(brow_sb[:], t["bpt_row"].ap())
        bfp_sb = const.tile([1, 138], BF)       # [b_f 128 | b_p 10]
        nc.sync.dma_start(bfp_sb[:], t["bfp_row"].ap())
        beba_sb = const.tile([128, 2], F32)     # col0 = 0.5*b_e, col1 = b_a
        nc.sync.dma_start(beba_sb[:], t["beba_col"].ap())
    e_bias = beba_sb[:, 0:1] if has_bias["ea"] else 0.0
    a_bias = beba_sb[:, 1:2] if has_bias["ea"] else 0.0

    nodes_ap = t["nodes"].ap()
    paths_ap = t["paths"].ap()
    table_aps = [nodes_ap, nodes_ap, paths_ap]
    out_ap = t["out"].ap()

    w_both = const.tile([128, 128], BF)  # [l, (b,m)] softmaxed w, both b's
    ctx = {}

    for b in range(BPC):
        # ---- q/r gathers: kT/vrT [d, l] bf16 ----
        kT = perb.tile([128, 128], BF, tag="kT", name=f"kT{b}")
        vrT = perb.tile([128, 128], BF, tag="vrT", name=f"vrT{b}")
        nc.gpsimd.dma_gather(
            kT[:].rearrange("p (o n) -> p o n", o=1), t["kemb"].ap(),
            idqr_sb[:, b * 8:(b + 1) * 8], 128, 128, E, transpose=True)
        nc.gpsimd.dma_gather(
            vrT[:].rearrange("p (o n) -> p o n", o=1), t["vemb"].ap(),
            idqr_sb[:, 16 + b * 8:16 + (b + 1) * 8], 128, 128, D, transpose=True)

        # ---- w = softmax(k @ Mk.T) -> w_both[:, b*64:(b+1)*64] ----
        w_ps = sps.tile([128, 256], F32, tag="sm", name=f"wps{b}")
        nc.tensor.matmul(w_ps[:, 0:64], kT[:], mkT_sb[:], start=True, stop=True)
        wneg = perb.tile([128, 1], F32, tag="wneg", name=f"wneg{b}")
        nc.vector.reduce_max(wneg[:], w_ps[:, 0:64], axis=AX.X, negate=True)
        wexp = perb.tile([128, 64], F32, tag="wexp", name=f"wexp{b}")
        wsum = perb.tile([128, 1], F32, tag="wsum", name=f"wsum{b}")
        nc.scalar.activation(wexp[:], w_ps[:, 0:64], AF.Exp, bias=wneg[:], accum_out=wsum[:])
        wrec = perb.tile([128, 1], F32, tag="wrec", name=f"wrec{b}")
        nc.vector.reciprocal(wrec[:], wsum[:])
        nc.vector.tensor_scalar_mul(w_both[:, b * 64:(b + 1) * 64], wexp[:], wrec[:])

        # ---- G gathers: fullT[k, (l,p)] per table, 2 half tiles of 64 l's ----
        G = []
        for tbl in range(3):
            halves = []
            for h in range(2):
                g = gpool.tile([128, 8192], BF, tag="G", name=f"G{b}{tbl}{h}")
                for ci in range(2):  # 4096 idx per call
                    col = tbl * 2048 + b * 1024 + h * 512 + ci * 256
                    nc.gpsimd.dma_gather(
                        g[:, ci * 4096:(ci + 1) * 4096].rearrange("p (o n) -> p o n", o=1),
                        table_aps[tbl], idg_sb[:, col:col + 256],
                        4096, 4096, E, transpose=True, single_packet=False)
                halves.append(g)
            G.append(halves)

        # ---- z -> tanh -> cw loop (2 l's per psum tile) ----
        cw_sb = perb.tile([128, 128], F32, tag="cw", name=f"cw{b}")
        for g0 in range(64):
            z_t = zps.tile([128, 2, 512], F32, tag="z", name=f"z{b}{g0}")
            for slot in range(2):
                l = g0 * 2 + slot
                h, lh = l // 64, l % 64
                for tbl in range(3):
                    nc.tensor.matmul(
                        z_t[:, slot, 0:384],
                        G[tbl][h][:, lh * 128:(lh + 1) * 128],
                        wpt_sb[:, tbl, :],
                        start=(tbl == 0), stop=(tbl == 2 and not has_bias["pt"]))
                if has_bias["pt"]:
                    nc.tensor.matmul(z_t[:, slot, 0:384], ones_bf[:],
                                     brow_sb[:, 0:384], start=False, stop=True)
            th = tanhp.tile([128, 2, 384], BF, tag="th", name=f"th{b}{g0}")
            nc.scalar.activation(th[:], z_t[:, :, 0:384], AF.Tanh)
            for slot in range(2):
                l = g0 * 2 + slot
                scr = scrp.tile([128, 384], BF, tag="cwscr", name=f"cs{b}{g0}{slot}")
                nc.vector.scalar_tensor_tensor(
                    scr[:], th[:, slot, :], 1.0, watt_rep[:],
                    OP.mult, OP.mult, accum_out=cw_sb[:, l:l + 1])

        # ---- attn = softmax over l (free dim of cw_sb[p, l]) ----
        aneg = perb.tile([128, 1], F32, tag="aneg", name=f"aneg{b}")
        nc.vector.reduce_max(aneg[:], cw_sb[:], axis=AX.X, negate=True)
        aexp = perb.tile([128, 128], F32, tag="aexp", name=f"aexp{b}")
        asum = perb.tile([128, 1], F32, tag="asum", name=f"asum{b}")
        nc.scalar.activation(aexp[:], cw_sb[:], AF.Exp, bias=aneg[:], accum_out=asum[:])
        arec = perb.tile([128, 1], F32, tag="arec", name=f"arec{b}")
        nc.vector.reciprocal(arec[:], asum[:])
        attn_bf = perb.tile([128, 128], BF, tag="attnbf", name=f"attnbf{b}")
        nc.vector.tensor_scalar_mul(attn_bf[:], aexp[:], arec[:])
        atr_ps = sps.tile([128, 512], BF, tag="sm", name=f"atrps{b}")
        nc.tensor.transpose(atr_ps[:, 0:128], attn_bf[:], ident_bf[:])
        attnT = perb.tile([128, 128], BF, tag="attnT", name=f"attnT{b}")
        nc.vector.tensor_copy(attnT[:], atr_ps[:, 0:128])

        # ---- phase 2: code_vecT[k, l] = sum_p fullT * attn ----
        cvT = perb.tile([128, 3, 128], BF, tag="cvT", name=f"cvT{b}")
        for q in range(4):  # quarters of 32 l's
            af = scrp.tile([1, 4096], BF, tag="flat", name=f"af{b}{q}", bufs=1)
            nc.sync.dma_start(af[0:1, :].rearrange("o (l p) -> o l p", p=128),
                              attnT[q * 32:(q + 1) * 32, :])
            arep = prodp.tile([128, 4096], BF, tag="arep", name=f"ar{b}{q}")
            nc.gpsimd.partition_broadcast(arep[:], af[0:1, :], channels=128)
            h, qh = q // 2, q % 2
            for tbl in range(3):
                prod = prodp.tile([128, 4096], BF, tag="prod", name=f"pr{b}{q}{tbl}", bufs=1)
                nc.vector.tensor_mul(prod[:], G[tbl][h][:, qh * 4096:(qh + 1) * 4096], arep[:])
                red = scrp.tile([128, 32], F32, tag="red", name=f"rd{b}{q}{tbl}")
                nc.vector.tensor_reduce(
                    red[:], prod[:].rearrange("p (l q) -> p l q", q=128),
                    axis=AX.X, op=OP.add)
                nc.vector.tensor_copy(cvT[:, tbl, q * 32:(q + 1) * 32], red[:])

        # ---- eT/aT: [ch, l] = W_half.T @ v (transposed orientation) ----
        eT_ps = sps.tile([128, 256], F32, tag="sm", name=f"etps{b}")
        aT_ps = sps.tile([128, 256], F32, tag="sm", name=f"atps{b}")
        for c in range(4):
            rhs = vrT[:] if c == 0 else cvT[:, c - 1, :]
            nc.tensor.matmul(eT_ps[:, 0:128], wea_sb[:, c, 0:128], rhs,
                             start=(c == 0), stop=(c == 3))
            nc.tensor.matmul(aT_ps[:, 0:128], wea_sb[:, c, 128:256], rhs,
                             start=(c == 0), stop=(c == 3))
        # e = sigmoid(x) = 0.5*tanh(0.5*x) + 0.5, clamped away from 0
        et = perb.tile([128, 128], F32, tag="et", name=f"et{b}")
        nc.scalar.activation(et[:], eT_ps[:, 0:128], AF.Tanh, scale=0.5, bias=e_bias)
        e0 = perb.tile([128, 128], F32, tag="e0", name=f"e0{b}")
        nc.vector.tensor_scalar(e0[:], et[:], 0.5, 0.5, OP.mult, OP.add)
        eT_sb = perb.tile([128, 128], F32, tag="eT", name=f"eTs{b}")
        nc.vector.tensor_scalar_max(eT_sb[:], e0[:], 1e-6)
        aT_sb = perb.tile([128, 128], F32, tag="aT", name=f"aTs{b}")
        nc.scalar.activation(aT_sb[:], aT_ps[:, 0:128], AF.Tanh, bias=a_bias)

        ctx[b] = dict(kT=kT, eT=eT_sb, aT=aT_sb)

    # ---- scan over l in [d, (b,m)] layout ----
    er = [const.tile([128, 128], F32, name=f"er{b}") for b in range(BPC)]
    u_full = scrp.tile([128, 128], F32, tag="u_full")
    v_full = scrp.tile([128, 128], F32, tag="v_full")
    for q in range(4):
        wfl = scrp.tile([1, 4096], BF, tag="flat", name=f"wfl{q}", bufs=1)
        nc.sync.dma_start(wfl[0:1, :].rearrange("o (l m) -> o l m", m=128),
                          w_both[q * 32:(q + 1) * 32, :])
        wtex = prodp.tile([128, 4096], BF, tag="wtex", name=f"wt{q}")
        nc.gpsimd.partition_broadcast(wtex[:], wfl[0:1, :], channels=128)
        for j in range(32):
            l = q * 32 + j
            for b in range(BPC):
                sl = slice(b * 64, (b + 1) * 64)
                wsl = wtex[:, j * 128 + b * 64: j * 128 + (b + 1) * 64]
                nc.vector.scalar_tensor_tensor(
                    u_full[:, sl], wsl, ctx[b]["eT"][:, l:l + 1], mv[:, sl],
                    OP.mult, OP.mult, accum_out=er[b][:, l:l + 1])
                nc.vector.scalar_tensor_tensor(
                    v_full[:, sl], wsl, ctx[b]["aT"][:, l:l + 1], u_full[:, sl],
                    OP.mult, OP.subtract)
            nc.vector.tensor_tensor(mv[:], mv[:], v_full[:], op=OP.add)

    # ---- read recovery + f + output per b ----
    for b in range(BPC):
        kT = ctx[b]["kT"]
        erec = perb.tile([128, 128], F32, tag="erec", name=f"erec{b}")
        nc.vector.reciprocal(erec[:], ctx[b]["eT"][:])
        readT = perb.tile([128, 128], BF, tag="readT", name=f"readT{b}")
        nc.vector.tensor_mul(readT[:], er[b][:], erec[:])

        f_ps = sps.tile([128, 256], F32, tag="sm", name=f"fps{b}")
        nc.tensor.matmul(f_ps[:, 0:128], readT[:], wf_sb[:, 0, :],
                         start=True, stop=False)
        nc.tensor.matmul(f_ps[:, 0:128], kT[:], wf_sb[:, 1, :],
                         start=False, stop=not has_bias["f"])
        if has_bias["f"]:
            nc.tensor.matmul(f_ps[:, 0:128], ones_bf[:], bfp_sb[:, 0:128],
                             start=False, stop=True)
        f_bf = perb.tile([128, 128], BF, tag="f_bf", name=f"fbf{b}")
        nc.scalar.activation(f_bf[:], f_ps[:, 0:128], AF.Tanh)
        ftr_ps = sps.tile([128, 512], BF, tag="sm", name=f"ftrps{b}")
        nc.tensor.transpose(ftr_ps[:, 0:128], f_bf[:], ident_bf[:])
        fT_bf = perb.tile([128, 128], BF, tag="fT", name=f"fT{b}")
        nc.vector.tensor_copy(fT_bf[:], ftr_ps[:, 0:128])
        o_ps = sps.tile([128, 256], F32, tag="sm", name=f"ops{b}")
        nc.tensor.matmul(o_ps[:, 0:10], fT_bf[:], wp_sb[:],
                         start=True, stop=not has_bias["p"])
        if has_bias["p"]:
            nc.tensor.matmul(o_ps[:, 0:10], ones_bf[:], bfp_sb[:, 128:138],
                             start=False, stop=True)
        ot = perb.tile([128, 10], F32, tag="ot", name=f"ot{b}")
        nc.scalar.activation(ot[:], o_ps[:, 0:10], AF.Tanh, scale=0.5)
        o_sb = perb.tile([128, 10], F32, tag="o_sb", name=f"osb{b}")
        nc.vector.tensor_scalar(o_sb[:], ot[:], 0.5, 0.5, OP.mult, OP.add)
        nc.sync.dma_start(out_ap[b, :, :], o_sb[:])

    for p in (sps, zps, prodp, perb, scrp, tanhp, gpool, const):
        p.release()


def _build(has_bias_key):
    nc = bacc.Bacc("TRN2", target_bir_lowering=False, debug=False)
    t = {}
    t["idx3"] = nc.dram_tensor("idx3", [128, 3 * 2048], I16, kind="ExternalInput")
    t["idxqr"] = nc.dram_tensor("idxqr", [128, 32], I16, kind="ExternalInput")
    t["nodes"] = nc.dram_tensor("nodes", [NODE_ROWS, E], BF, kind="ExternalInput")
    t["paths"] = nc.dram_tensor("paths", [PATH_ROWS, E], BF, kind="ExternalInput")
    t["kemb"] = nc.dram_tensor("kemb", [NUM_C + 1, D], BF, kind="ExternalInput")
    t["vemb"] = nc.dram_tensor("vemb", [2 * NUM_C + 2, D], BF, kind="ExternalInput")
    t["wpt"] = nc.dram_tensor("wpt", [384, 384], BF, kind="ExternalInput")
    t["watt_rep"] = nc.dram_tensor("watt_rep", [128, 384], BF, kind="ExternalInput")
    t["wea"] = nc.dram_tensor("wea", [512, 256], BF, kind="ExternalInput")
    t["wf"] = nc.dram_tensor("wf", [256, 128], BF, kind="ExternalInput")
    t["wp"] = nc.dram_tensor("wp", [128, 10], BF, kind="ExternalInput")
    t["mkT"] = nc.dram_tensor("mkT", [128, 64], BF, kind="ExternalInput")
    t["mv0T2"] = nc.dram_tensor("mv0T2", [128, 128], F32, kind="ExternalInput")
    t["ident_bf"] = nc.dram_tensor("ident_bf", [128, 128], BF, kind="ExternalInput")
    t["ones_row"] = nc.dram_tensor("ones_row", [1, 128], BF, kind="ExternalInput")
    t["bpt_row"] = nc.dram_tensor("bpt_row", [1, 384], BF, kind="ExternalInput")
    t["bfp_row"] = nc.dram_tensor("bfp_row", [1, 138], BF, kind="ExternalInput")
    t["beba_col"] = nc.dram_tensor("beba_col", [128, 2], F32, kind="ExternalInput")
    t["out"] = nc.dram_tensor("out", [BPC, L, 10], F32, kind="ExternalOutput")

    has_bias = dict(has_bias_key)
    with tile.TileContext(nc) as tc:
        t["tc"] = tc
        _emit(nc, t, has_bias)
    nc.compile()
    return nc


def _prep_inputs(code, q, r, embed_nodes, embed_paths, W_pt, b_pt, W_att, b_att,
                 k_emb, Mk, Mv0, v_emb, W_e, b_e, W_a, b_a, W_f, b_f, W_p, b_p):
    """Host-side layout prep (dtype casts / transposes / index wrapping only)."""
    code = np.asarray(code)
    c2v = code[:, :, NUM_C * 2:].reshape(B, L, P, 3)
    idx_comp = [c2v[..., 0], c2v[..., 2], c2v[..., 1]]  # full = [nodes[i0], nodes[i2], paths[i1]]

    nodes_bf = np.asarray(embed_nodes).astype(BF16)
    paths_bf = np.asarray(embed_paths).astype(BF16)
    kemb_bf = np.asarray(k_emb).astype(BF16)
    vemb_bf = np.asarray(v_emb).astype(BF16)
    wpt_bf = np.asarray(W_pt).astype(BF16)
    watt_rep = np.tile(np.asarray(W_att).reshape(1, 384), (128, 1)).astype(BF16)
    wea_bf = np.concatenate([np.asarray(W_e), np.asarray(W_a)], axis=1).astype(BF16)
    wf_bf = np.asarray(W_f).astype(BF16)
    wp_bf = np.asarray(W_p).astype(BF16)
    mkT_bf = np.ascontiguousarray(np.asarray(Mk).T).astype(BF16)
    mv0T2 = np.concatenate([np.asarray(Mv0).T, np.asarray(Mv0).T], axis=1).astype(np.float32)
    ident_bf = np.eye(128, dtype=BF16)
    ones_row = np.ones((1, 128), dtype=BF16)
    bpt_row = np.asarray(b_pt).reshape(1, 384).astype(BF16)
    bfp_row = np.zeros((1, 138), dtype=np.float32)
    bfp_row[0, 0:128] = np.asarray(b_f)
    bfp_row[0, 128:138] = np.asarray(b_p)
    bfp_row = bfp_row.astype(BF16)
    beba_col = np.zeros((128, 2), dtype=np.float32)
    beba_col[:, 0] = 0.5 * np.asarray(b_e)
    beba_col[:, 1] = np.asarray(b_a)

    has_bias = {
        "pt": bool(np.any(np.asarray(b_pt))),
        "ea": bool(np.any(np.asarray(b_e))) or bool(np.any(np.asarray(b_a))),
        "f": bool(np.any(np.asarray(b_f))),
        "p": bool(np.any(np.asarray(b_p))),
    }
    has_bias["any"] = any(has_bias.values())
    # b_att shifts every logit equally -> cancelled by the softmax; no term.

    in_maps = []
    for c in range(CORES):
        bs = slice(c * BPC, (c + 1) * BPC)
        idx3 = np.concatenate(
            [_wrap_idx(np.ascontiguousarray(comp[bs]).reshape(-1)) for comp in idx_comp],
            axis=1)
        qf = np.ascontiguousarray(np.asarray(q)[bs]).reshape(-1)
        rf = np.ascontiguousarray(np.asarray(r)[bs]).reshape(-1)
        idxqr = np.concatenate([_wrap_idx(qf), _wrap_idx(rf)], axis=1)
        in_maps.append({
            "idx3": idx3, "idxqr": idxqr,
            "nodes": nodes_bf, "paths": paths_bf,
            "kemb": kemb_bf, "vemb": vemb_bf,
            "wpt": wpt_bf, "watt_rep": watt_rep, "wea": wea_bf,
            "wf": wf_bf, "wp": wp_bf, "mkT": mkT_bf, "mv0T2": mv0T2,
            "ident_bf": ident_bf, "ones_row": ones_row,
            "bpt_row": bpt_row, "bfp_row": bfp_row, "beba_col": beba_col,
        })
    return in_maps, has_bias


def get_nc_and_inputs(**inputs):
    in_maps, has_bias = _prep_inputs(**inputs)
    key = tuple(sorted((k, v) for k, v in has_bias.items()))
    if key not in _CACHE:
        _CACHE[key] = _build(key)
    return _CACHE[key], in_maps


def kernel(**inputs):
    nc, in_maps = get_nc_and_inputs(**inputs)
    res = run_bass_kernel_spmd(nc, in_maps, core_ids=list(range(CORES)),
                               trace=bool(int(os.environ.get("KERNEL_TRACE", "0"))))
    if res.exec_time_ns is not None:
        kernel.last_exec_time_ns = res.exec_time_ns
    out = np.concatenate([res.results[c]["out"] for c in range(CORES)], axis=0)
    return out.astype(np.float32)
